# revision 6
# baseline (speedup 1.0000x reference)
"""Differential multi-head attention Trainium2 Bass kernel (v3).

Problem: B=4, N=1024, D=512, H=8 heads, DH=64. LAM=0.5.
  q = (x@Wq+bq)  -> [B,H,N,2*DH], halves q1,q2 (same for k)
  a_i = softmax(q_i@k_i^T / sqrt(DH)); attn = a1 - LAM*a2; out = attn@v

Sharding: 8 cores; core c handles batch b=c//2 and heads h0..h0+3 with
h0=(c%2)*4 (batch + head-group parallel). Weights column-sharded by head.

Design: ACT (exp over 8M elems/core, ~67us busy) is the roofline.
  - scores [k,q]-transposed per (half, kt) in [128,1024] psum tiles; the
    two halves' K=64 matmuls sit on disjoint PE row groups (partitions
    0-63 / 64-127) and are emitted adjacently for HW row packing.
  - exp via one ACTIVATE per (half, kt); PV accumulates e@vaug with the
    [+1 | v | -2] 66-col merged block per head (halves share v columns,
    denominators ride along as row 0 / row 64 of u).
  - PV of head h-1 and proj of head h+1 are interleaved into head h's
    score emission so PE never outruns/starves ACT.
  - finish: PE transpose -> reciprocal_approx_fast -> tensor_scalar mul
    + fused scalar_tensor_tensor; last pair pipelined qc-major to keep
    the kernel tail short.
"""
import sys

sys.path.insert(0, "/opt/trn_rl_repo")

from contextlib import ExitStack

import numpy as np

import concourse.bass as bass
import concourse.mybir as mybir
import concourse.tile as tile
from concourse import bacc, bass_utils
from concourse.masks import make_identity

F32 = mybir.dt.float32
BF16 = mybir.dt.bfloat16

B, N, D, H = 4, 1024, 512, 8
DH = 64            # per-head dim for v and per q/k half
HPC = 4            # heads per core
LAM = 0.5
SCALE = 0.125      # 1/sqrt(DH)
NCORES = 8
CQ = HPC * 2 * DH  # 512 projection cols per core for q/k
CV = HPC * DH      # 256 projection cols per core for v
P = 128
NT = N // P        # 8 seq tiles
DC = D // P        # 4 contraction chunks
QW = 512           # query chunk width (psum bank limit for fp32)
QC = N // QW       # 2 query chunks
AUG = DH + 1       # 65: v cols + one constant col per half
GW = DH + 2        # 66: per-head vaug group [+1 | v(64) | -2]
XWDT = BF16
QKDT = BF16
EDT = BF16
NB = 2 * HPC + CV  # combined bias tensor cols


def build_nc(reps=1):
    nc = bacc.Bacc("TRN2", target_bir_lowering=False, debug=False,
                   num_devices=NCORES)
    d = {
        "xt": nc.dram_tensor("xt", [D, N], XWDT, kind="ExternalInput"),
        "wq": nc.dram_tensor("wq", [D, CQ], XWDT, kind="ExternalInput"),
        "wk": nc.dram_tensor("wk", [D, CQ], XWDT, kind="ExternalInput"),
        "wv": nc.dram_tensor("wv", [D, CV], XWDT, kind="ExternalInput"),
        "bias": nc.dram_tensor("bias", [P, NB], F32, kind="ExternalInput"),
        "o": nc.dram_tensor("o", [N, CV], F32, kind="ExternalOutput"),
    }
    with tile.TileContext(nc) as tc, ExitStack() as ctx:
        consts = ctx.enter_context(tc.tile_pool(name="consts", bufs=1))
        qk = ctx.enter_context(tc.tile_pool(name="qk", bufs=2))
        vaugp = ctx.enter_context(tc.tile_pool(name="vaugp", bufs=1))
        ep = ctx.enter_context(tc.tile_pool(name="ep", bufs=26))
        up = ctx.enter_context(tc.tile_pool(name="up", bufs=6))
        outp = ctx.enter_context(tc.tile_pool(name="outp", bufs=1))
        smallp = ctx.enter_context(tc.tile_pool(name="smallp", bufs=4))
        # 8 PSUM banks: scores 3x2, pv 1 (also v-proj), misc 1 (q/k proj +
        # transposes -- never overlapping in schedule position)
        ps_misc = ctx.enter_context(
            tc.tile_pool(name="ps_misc", bufs=1, space="PSUM"))
        ps_score = ctx.enter_context(
            tc.tile_pool(name="ps_score", bufs=3, space="PSUM"))
        ps_pv = ctx.enter_context(
            tc.tile_pool(name="ps_pv", bufs=1, space="PSUM"))

        def body():
            # ---- input DMAs (few, large; head-pair weight chunks so head 0
            # can start early)
            xt_t = consts.tile([P, DC, N], XWDT, tag="xt", name="xt")
            nc.sync.dma_start(
                xt_t[:],
                d["xt"][:].rearrange("(dc p) n -> p dc n", p=P))
            xt_sb = [xt_t[:, dc, :] for dc in range(DC)]

            def load_w(name, cols, parts):
                t = consts.tile([P, DC, cols], XWDT, tag=name, name=name)
                src = d[name][:].rearrange("(dc p) c -> p dc c", p=P)
                cw = cols // parts
                for i in range(parts):
                    nc.sync.dma_start(t[:, :, i * cw:(i + 1) * cw],
                                      src[:, :, i * cw:(i + 1) * cw])
                return t

            wq_t = load_w("wq", CQ, 2)
            wk_t = load_w("wk", CQ, 2)
            wv_t = load_w("wv", CV, 1)
            bias_sb = consts.tile([P, NB], F32, tag="bias", name="bias")
            nc.sync.dma_start(bias_sb[:], d["bias"][:])
            bq_sb = bias_sb[:, 0:HPC]
            bk_sb = bias_sb[:, HPC:2 * HPC]
            bvb_sb = bias_sb[:, 2 * HPC:]
            ident = consts.tile([P, P], F32, tag="ident", name="ident")
            make_identity(nc, ident[:])

            # exp table preload: tiny activation early
            warm = consts.tile([P, 1], F32, tag="warm", name="warm")
            nc.vector.memset(warm[:], 0.0)
            nc.scalar.activation(warm[:], warm[:],
                                 mybir.ActivationFunctionType.Exp)

            # ---- merged vaug tile: [128, NT, HPC, 66] = [+1 | v | -2]
            vaug = vaugp.tile([P, NT, HPC, GW], EDT, tag="vaug", name="vaug")
            nc.vector.memset(vaug[:, :, :, 0:1], 1.0)
            nc.vector.memset(vaug[:, :, :, GW - 1:GW], -2.0)

            def proj_chunk(h, w_t, b_sb, dest, pfx, qc, pool, tag):
                ps = pool.tile([P, QW], F32, tag=tag,
                               name=f"ps_{pfx}{h}_{qc}")
                for dc in range(DC):
                    nc.tensor.matmul(
                        ps[:],
                        w_t[:, dc, h * P:(h + 1) * P],
                        xt_sb[dc][:, qc * QW:(qc + 1) * QW],
                        start=(dc == 0), stop=(dc == DC - 1))
                nc.vector.tensor_scalar_add(
                    dest[:, qc * QW:(qc + 1) * QW], ps[:], b_sb[:, h:h + 1])

            def proj_head_q(h, pool=None, tag=None):
                qt = qk.tile([P, N], QKDT, tag="qt", name=f"qt{h}")
                for qc in range(QC):
                    proj_chunk(h, wq_t, bq_sb, qt, "q", qc,
                               pool or ps_misc, tag or "misc")
                return qt

            def proj_head_k(h, pool=None, tag=None):
                kt_ = qk.tile([P, N], QKDT, tag="kt", name=f"kt{h}")
                for qc in range(QC):
                    proj_chunk(h, wk_t, bk_sb, kt_, "k", qc,
                               pool or ps_misc, tag or "misc")
                return kt_

            def proj_v(nt):
                # v projection psum borrows the pv pool (idle during head 0)
                ps = ps_pv.tile([P, QW], F32, tag="pv", name=f"ps_v{nt}")
                for dc in range(DC):
                    nc.tensor.matmul(
                        ps[0:P, 0:CV],
                        xt_sb[dc][:, nt * P:(nt + 1) * P],
                        wv_t[:, dc, :],
                        start=(dc == 0), stop=(dc == DC - 1))
                psv = ps[0:P, 0:CV].rearrange("p (h a) -> p h a", a=DH)
                bvv = bvb_sb.rearrange("p (h a) -> p h a", a=DH)
                nc.vector.tensor_add(vaug[:, nt, :, 1:1 + DH], psv, bvv)

            score_ps = {}

            def score_mm_qc(h, qt, kt_, kt, es, qc):
                # one qc chunk of both halves' score matmuls + split exp;
                # used in the prologue so exp starts before q/k qc1 exist
                if (h, kt) not in score_ps:
                    score_ps[(h, kt)] = [
                        ps_score.tile([P, N], F32, tag="score",
                                      name=f"ps_s{h}_{kt}_{half}")
                        for half in range(2)]
                    for half in range(2):
                        es[(half, kt)] = ep.tile(
                            [P, N], EDT, tag="e", name=f"e{h}_{kt}_{half}")
                ps = score_ps[(h, kt)]
                for half in range(2):
                    lhsT = kt_[half * DH:(half + 1) * DH,
                               kt * P:(kt + 1) * P]
                    rhs = qt[half * DH:(half + 1) * DH,
                             qc * QW:(qc + 1) * QW]
                    nc.tensor.matmul(
                        ps[half][:, qc * QW:(qc + 1) * QW], lhsT, rhs,
                        start=True, stop=True)
                for half in range(2):
                    sl = slice(qc * QW, (qc + 1) * QW)
                    nc.scalar.activation(es[(half, kt)][:, sl],
                                         ps[half][:, sl],
                                         mybir.ActivationFunctionType.Exp,
                                         scale=SCALE)

            def score_exp_kt(h, qt, kt_, kt, es):
                ps = [ps_score.tile([P, N], F32, tag="score",
                                    name=f"ps_s{h}_{kt}_{half}")
                      for half in range(2)]
                # interleave halves: disjoint PE row groups pack on HW
                for qc in range(QC):
                    for half in range(2):
                        lhsT = kt_[half * DH:(half + 1) * DH,
                                   kt * P:(kt + 1) * P]
                        rhs = qt[half * DH:(half + 1) * DH,
                                 qc * QW:(qc + 1) * QW]
                        nc.tensor.matmul(
                            ps[half][:, qc * QW:(qc + 1) * QW], lhsT, rhs,
                            start=True, stop=True)
                for half in range(2):
                    e = ep.tile([P, N], EDT, tag="e",
                                name=f"e{h}_{kt}_{half}")
                    nc.scalar.activation(e[:], ps[half][:],
                                         mybir.ActivationFunctionType.Exp,
                                         scale=SCALE)
                    es[(half, kt)] = e

            def pv_piece(h, es, us, qc, half, pool=None, tag=None,
                         copy_act=False):
                u = us[half]
                ps = (pool or ps_pv).tile([AUG, QW], F32, tag=tag or "pv",
                                          name=f"ps_pv{h}_{half}_{qc}")
                for kt in range(NT):
                    nc.tensor.matmul(
                        ps[:],
                        vaug[:, kt, h, half:half + AUG],
                        es[(half, kt)][:, qc * QW:(qc + 1) * QW],
                        start=(kt == 0), stop=(kt == NT - 1))
                if copy_act:
                    nc.scalar.activation(u[:, qc * QW:(qc + 1) * QW], ps[:],
                                         mybir.ActivationFunctionType.Copy)
                else:
                    nc.vector.tensor_copy(u[:, qc * QW:(qc + 1) * QW], ps[:])

            ostage = []
            for qt_i in range(NT):
                t = outp.tile([P, CV], F32, tag=f"ost{qt_i}", name=f"ost{qt_i}")
                ostage.append(t)

            def finish_pair(hs, us_pair, qt_i, pool=None, tag=None,
                            use_act=False):
                # transpose u blocks: [h0u1 | h0u2 | h1u1 | h1u2], 65 cols
                tr = (pool or ps_misc).tile([P, 4 * AUG], F32,
                                            tag=tag or "misc",
                                            name=f"ps_tr{hs[0]}_{qt_i}")
                for j, h in enumerate(hs):
                    u1, u2 = us_pair[h]
                    nc.tensor.transpose(
                        tr[:, (2 * j) * AUG:(2 * j + 1) * AUG],
                        u1[0:AUG, qt_i * P:(qt_i + 1) * P],
                        ident[0:AUG, 0:AUG])
                    nc.tensor.transpose(
                        tr[:, (2 * j + 1) * AUG:(2 * j + 2) * AUG],
                        u2[0:AUG, qt_i * P:(qt_i + 1) * P],
                        ident[0:AUG, 0:AUG])
                # denominators at cols {0, 129, 130, 259} = [(130,2),(129,2)]
                rr = smallp.tile([P, 4], F32, tag="rr",
                                 name=f"rr_{hs[0]}{qt_i}")
                den = tr[:].rearrange(
                    "p (a b) -> p a b", b=2 * AUG)[:, :, ::2 * AUG - 1]
                nc.vector.reciprocal_approx_fast(
                    rr[:].rearrange("p (a b) -> p a b", b=2), den)
                for j, h in enumerate(hs):
                    o1 = smallp.tile([P, DH], F32, tag="o1",
                                     name=f"o1_{h}{qt_i}")
                    if use_act:
                        nc.scalar.activation(
                            o1[:],
                            tr[:, (2 * j) * AUG + 1:(2 * j) * AUG + AUG],
                            mybir.ActivationFunctionType.Copy,
                            scale=rr[:, 2 * j:2 * j + 1])
                    else:
                        nc.vector.tensor_scalar_mul(
                            o1[:],
                            tr[:, (2 * j) * AUG + 1:(2 * j) * AUG + AUG],
                            rr[:, 2 * j:2 * j + 1])
                    nc.vector.scalar_tensor_tensor(
                        ostage[qt_i][:, h * DH:(h + 1) * DH],
                        tr[:, (2 * j + 1) * AUG:(2 * j + 1) * AUG + DH],
                        rr[:, 2 * j + 1:2 * j + 2],
                        o1[:],
                        op0=mybir.AluOpType.mult,
                        op1=mybir.AluOpType.add)

            # ---------------- schedule ----------------
            # head-0 ramp: qc0 proj chunks -> first score MMs + split exps
            # while the qc1 chunks are still projecting
            qt0 = qk.tile([P, N], QKDT, tag="qt", name="qt0")
            kt0 = qk.tile([P, N], QKDT, tag="kt", name="kt0")
            es0 = {}
            proj_chunk(0, wq_t, bq_sb, qt0, "q", 0, ps_misc, "misc")
            proj_chunk(0, wk_t, bk_sb, kt0, "k", 0, ps_pv, "pv")
            RAMP = 2
            for kt in range(RAMP):
                score_mm_qc(0, qt0, kt0, kt, es0, 0)
            proj_chunk(0, wq_t, bq_sb, qt0, "q", 1, ps_misc, "misc")
            proj_chunk(0, wk_t, bk_sb, kt0, "k", 1, ps_pv, "pv")
            for kt in range(RAMP):
                score_mm_qc(0, qt0, kt0, kt, es0, 1)
            heads = {0: (qt0, kt0)}
            all_es = {}
            all_us = {}

            def emit_out_dma(qt_i):
                nc.sync.dma_start(d["o"][qt_i * P:(qt_i + 1) * P, :],
                                  ostage[qt_i][:])

            # pv piece order per head: qc-major so finish can start early
            PIECES = [(0, 0), (0, 1), (1, 0), (1, 1)]

            for h in range(HPC):
                qt, kt_ = heads[h]
                es = es0 if h == 0 else {}
                all_es[h] = es
                if h >= 1:
                    all_us[h - 1] = [
                        up.tile([AUG, N], F32, tag="u", name=f"u{h - 1}_{hf}")
                        for hf in range(2)]
                for kt in range(NT):
                    if not (h == 0 and kt < 2):
                        score_exp_kt(h, qt, kt_, kt, es)
                    if h == 0:
                        # v projection spread through head 0's score phase
                        if kt % 2 == 1:
                            proj_v(kt - 1)
                            proj_v(kt)
                    elif kt % 2 == 1:
                        # one PV piece of head h-1 every other kt
                        qc, half = PIECES[kt // 2]
                        pv_piece(h - 1, all_es[h - 1], all_us[h - 1],
                                 qc, half)
                    # finishes of pair (0,1): qt 0-3 spread over h=2 kt 4-7
                    # (qc0 pieces of pv(1) land by kt=3), qt 4-7 over h=3
                    if h == 2 and kt >= 4:
                        finish_pair((0, 1), all_us, kt - 4)
                    if h == 3 and kt < 4:
                        finish_pair((0, 1), all_us, kt + 4)
                    if kt == 0 and h + 1 < HPC:
                        heads[h + 1] = (proj_head_q(h + 1), None)
                    if kt == 2 and h + 1 < HPC:
                        heads[h + 1] = (heads[h + 1][0], proj_head_k(h + 1))

            # tail: pv of last head, finish last pair qc-pipelined
            h = HPC - 1
            all_us[h] = [up.tile([AUG, N], F32, tag="u", name=f"u{h}_{hf}")
                         for hf in range(2)]
            for qc in range(QC):
                pv_piece(h, all_es[h], all_us[h], qc, 0)
                pv_piece(h, all_es[h], all_us[h], qc, 1,
                         pool=ps_score, tag="score")
                lo, hi = (0, NT // 2) if qc == 0 else (NT // 2, NT)
                for i, qt_i in enumerate(range(lo, hi)):
                    if i % 2 == 0:
                        finish_pair((h - 1, h), all_us, qt_i, use_act=True)
                    else:
                        finish_pair((h - 1, h), all_us, qt_i,
                                    pool=ps_score, tag="score",
                                    use_act=True)
                    emit_out_dma(qt_i)

        if reps == 1:
            body()
        else:
            tc.For_i_unrolled_general(
                0, reps, 1,
                lambda iv0, unroll: [body() for _ in range(unroll)],
                max_unroll=2,
                hint_engines=(mybir.EngineType.PE, mybir.EngineType.DVE))

    nc.compile()
    return nc


_NC_CACHE = {}


def get_nc(reps=1):
    if reps not in _NC_CACHE:
        _NC_CACHE[reps] = build_nc(reps)
    return _NC_CACHE[reps]


def shard_inputs(inputs):
    import ml_dtypes
    xw_np = np.dtype(ml_dtypes.bfloat16)
    x = np.asarray(inputs["x"], dtype=np.float32)
    Wq = np.asarray(inputs["Wq"], dtype=np.float32)
    bq = np.asarray(inputs["bq"], dtype=np.float32)
    Wk = np.asarray(inputs["Wk"], dtype=np.float32)
    bk = np.asarray(inputs["bk"], dtype=np.float32)
    Wv = np.asarray(inputs["Wv"], dtype=np.float32)
    bv = np.asarray(inputs["bv"], dtype=np.float32)
    in_maps = []
    for c in range(NCORES):
        b = c // 2
        h0 = (c % 2) * HPC
        cq0 = h0 * 2 * DH
        cv0 = h0 * DH
        bias = np.zeros((P, NB), dtype=np.float32)
        bias[:, 0:HPC] = bq[cq0:cq0 + CQ].reshape(HPC, P).T
        bias[:, HPC:2 * HPC] = bk[cq0:cq0 + CQ].reshape(HPC, P).T
        bias[:, 2 * HPC:] = np.broadcast_to(bv[cv0:cv0 + CV], (P, CV))
        in_maps.append({
            "xt": np.ascontiguousarray(x[b].T).astype(xw_np),
            "wq": np.ascontiguousarray(Wq[:, cq0:cq0 + CQ]).astype(xw_np),
            "wk": np.ascontiguousarray(Wk[:, cq0:cq0 + CQ]).astype(xw_np),
            "wv": np.ascontiguousarray(Wv[:, cv0:cv0 + CV]).astype(xw_np),
            "bias": bias,
        })
    return in_maps


def assemble_output(results):
    out = np.empty((B, N, D), dtype=np.float32)
    for c in range(NCORES):
        b = c // 2
        g = c % 2
        out[b, :, g * CV:(g + 1) * CV] = results[c]["o"]
    return out


def kernel(**inputs):
    nc = get_nc(1)
    in_maps = shard_inputs(inputs)
    res = bass_utils.run_bass_kernel_spmd(
        nc, in_maps, core_ids=list(range(NCORES)))
    return assemble_output(res.results)


# revision 7
# speedup vs baseline: 1.0247x; 1.0247x over previous
"""Differential multi-head attention Trainium2 Bass kernel (v3).

Problem: B=4, N=1024, D=512, H=8 heads, DH=64. LAM=0.5.
  q = (x@Wq+bq)  -> [B,H,N,2*DH], halves q1,q2 (same for k)
  a_i = softmax(q_i@k_i^T / sqrt(DH)); attn = a1 - LAM*a2; out = attn@v

Sharding: 8 cores; core c handles batch b=c//2 and heads h0..h0+3 with
h0=(c%2)*4 (batch + head-group parallel). Weights column-sharded by head.

Design: ACT (exp over 8M elems/core, ~67us busy) is the roofline.
  - scores [k,q]-transposed per (half, kt) in [128,1024] psum tiles; the
    two halves' K=64 matmuls sit on disjoint PE row groups (partitions
    0-63 / 64-127) and are emitted adjacently for HW row packing.
  - exp via one ACTIVATE per (half, kt); PV accumulates e@vaug with the
    [+1 | v | -2] 66-col merged block per head (halves share v columns,
    denominators ride along as row 0 / row 64 of u).
  - PV of head h-1 and proj of head h+1 are interleaved into head h's
    score emission so PE never outruns/starves ACT.
  - finish: PE transpose -> reciprocal_approx_fast -> tensor_scalar mul
    + fused scalar_tensor_tensor; last pair pipelined qc-major to keep
    the kernel tail short.
"""
import sys

sys.path.insert(0, "/opt/trn_rl_repo")

from contextlib import ExitStack

import numpy as np

import concourse.bass as bass
import concourse.mybir as mybir
import concourse.tile as tile
from concourse import bacc, bass_utils
from concourse.masks import make_identity

F32 = mybir.dt.float32
BF16 = mybir.dt.bfloat16

B, N, D, H = 4, 1024, 512, 8
DH = 64            # per-head dim for v and per q/k half
HPC = 4            # heads per core
LAM = 0.5
SCALE = 0.125      # 1/sqrt(DH)
NCORES = 8
CQ = HPC * 2 * DH  # 512 projection cols per core for q/k
CV = HPC * DH      # 256 projection cols per core for v
P = 128
NT = N // P        # 8 seq tiles
DC = D // P        # 4 contraction chunks
QW = 512           # query chunk width (psum bank limit for fp32)
QC = N // QW       # 2 query chunks
AUG = DH + 1       # 65: v cols + one constant col per half
GW = DH + 2        # 66: per-head vaug group [+1 | v(64) | -2]
XWDT = BF16
QKDT = BF16
EDT = BF16
NB = 2 * HPC + CV  # combined bias tensor cols


def build_nc(reps=1):
    nc = bacc.Bacc("TRN2", target_bir_lowering=False, debug=False,
                   num_devices=NCORES)
    d = {
        "xt": nc.dram_tensor("xt", [D, N], XWDT, kind="ExternalInput"),
        "wq": nc.dram_tensor("wq", [D, CQ], XWDT, kind="ExternalInput"),
        "wk": nc.dram_tensor("wk", [D, CQ], XWDT, kind="ExternalInput"),
        "wv": nc.dram_tensor("wv", [D, CV], XWDT, kind="ExternalInput"),
        "bias": nc.dram_tensor("bias", [P, NB], F32, kind="ExternalInput"),
        "o": nc.dram_tensor("o", [N, CV], F32, kind="ExternalOutput"),
    }
    with tile.TileContext(nc) as tc, ExitStack() as ctx:
        consts = ctx.enter_context(tc.tile_pool(name="consts", bufs=1))
        qk = ctx.enter_context(tc.tile_pool(name="qk", bufs=2))
        vaugp = ctx.enter_context(tc.tile_pool(name="vaugp", bufs=1))
        ep = ctx.enter_context(tc.tile_pool(name="ep", bufs=26))
        up = ctx.enter_context(tc.tile_pool(name="up", bufs=6))
        outp = ctx.enter_context(tc.tile_pool(name="outp", bufs=1))
        smallp = ctx.enter_context(tc.tile_pool(name="smallp", bufs=4))
        # 8 PSUM banks: scores 3x2, pv 1 (also v-proj), misc 1 (q/k proj +
        # transposes -- never overlapping in schedule position)
        ps_misc = ctx.enter_context(
            tc.tile_pool(name="ps_misc", bufs=1, space="PSUM"))
        ps_score = ctx.enter_context(
            tc.tile_pool(name="ps_score", bufs=3, space="PSUM"))
        ps_pv = ctx.enter_context(
            tc.tile_pool(name="ps_pv", bufs=1, space="PSUM"))

        def body():
            # ---- input DMAs (few, large; head-pair weight chunks so head 0
            # can start early)
            xt_t = consts.tile([P, DC, N], XWDT, tag="xt", name="xt")
            nc.sync.dma_start(
                xt_t[:],
                d["xt"][:].rearrange("(dc p) n -> p dc n", p=P))
            xt_sb = [xt_t[:, dc, :] for dc in range(DC)]

            def load_w(name, cols, parts):
                t = consts.tile([P, DC, cols], XWDT, tag=name, name=name)
                src = d[name][:].rearrange("(dc p) c -> p dc c", p=P)
                cw = cols // parts
                for i in range(parts):
                    nc.sync.dma_start(t[:, :, i * cw:(i + 1) * cw],
                                      src[:, :, i * cw:(i + 1) * cw])
                return t

            wq_t = load_w("wq", CQ, 2)
            wk_t = load_w("wk", CQ, 2)
            wv_t = load_w("wv", CV, 1)
            bias_sb = consts.tile([P, NB], F32, tag="bias", name="bias")
            nc.sync.dma_start(bias_sb[:], d["bias"][:])
            bq_sb = bias_sb[:, 0:HPC]
            bk_sb = bias_sb[:, HPC:2 * HPC]
            bvb_sb = bias_sb[:, 2 * HPC:]
            ident = consts.tile([P, P], F32, tag="ident", name="ident")
            make_identity(nc, ident[:])

            # exp table preload: tiny activation early
            warm = consts.tile([P, 1], F32, tag="warm", name="warm")
            nc.vector.memset(warm[:], 0.0)
            nc.scalar.activation(warm[:], warm[:],
                                 mybir.ActivationFunctionType.Exp)

            # ---- merged vaug tile: [128, NT, HPC, 66] = [+1 | v | -2]
            vaug = vaugp.tile([P, NT, HPC, GW], EDT, tag="vaug", name="vaug")
            nc.vector.memset(vaug[:, :, :, 0:1], 1.0)
            nc.vector.memset(vaug[:, :, :, GW - 1:GW], -2.0)

            def proj_chunk(h, w_t, b_sb, dest, pfx, qc, pool, tag):
                ps = pool.tile([P, QW], F32, tag=tag,
                               name=f"ps_{pfx}{h}_{qc}")
                for dc in range(DC):
                    nc.tensor.matmul(
                        ps[:],
                        w_t[:, dc, h * P:(h + 1) * P],
                        xt_sb[dc][:, qc * QW:(qc + 1) * QW],
                        start=(dc == 0), stop=(dc == DC - 1))
                nc.vector.tensor_scalar_add(
                    dest[:, qc * QW:(qc + 1) * QW], ps[:], b_sb[:, h:h + 1])

            def proj_head_q(h, pool=None, tag=None):
                qt = qk.tile([P, N], QKDT, tag="qt", name=f"qt{h}")
                for qc in range(QC):
                    proj_chunk(h, wq_t, bq_sb, qt, "q", qc,
                               pool or ps_misc, tag or "misc")
                return qt

            def proj_head_k(h, pool=None, tag=None):
                kt_ = qk.tile([P, N], QKDT, tag="kt", name=f"kt{h}")
                for qc in range(QC):
                    proj_chunk(h, wk_t, bk_sb, kt_, "k", qc,
                               pool or ps_misc, tag or "misc")
                return kt_

            def proj_v(nt):
                # v projection psum borrows the pv pool (idle during head 0)
                ps = ps_pv.tile([P, QW], F32, tag="pv", name=f"ps_v{nt}")
                for dc in range(DC):
                    nc.tensor.matmul(
                        ps[0:P, 0:CV],
                        xt_sb[dc][:, nt * P:(nt + 1) * P],
                        wv_t[:, dc, :],
                        start=(dc == 0), stop=(dc == DC - 1))
                psv = ps[0:P, 0:CV].rearrange("p (h a) -> p h a", a=DH)
                bvv = bvb_sb.rearrange("p (h a) -> p h a", a=DH)
                nc.vector.tensor_add(vaug[:, nt, :, 1:1 + DH], psv, bvv)

            score_ps = {}

            def score_mm_qc(h, qt, kt_, kt, es, qc):
                # one qc chunk of both halves' score matmuls + split exp;
                # used in the prologue so exp starts before q/k qc1 exist
                if (h, kt) not in score_ps:
                    score_ps[(h, kt)] = [
                        ps_score.tile([P, N], F32, tag="score",
                                      name=f"ps_s{h}_{kt}_{half}")
                        for half in range(2)]
                    for half in range(2):
                        es[(half, kt)] = ep.tile(
                            [P, N], EDT, tag="e", name=f"e{h}_{kt}_{half}")
                ps = score_ps[(h, kt)]
                for half in range(2):
                    lhsT = kt_[half * DH:(half + 1) * DH,
                               kt * P:(kt + 1) * P]
                    rhs = qt[half * DH:(half + 1) * DH,
                             qc * QW:(qc + 1) * QW]
                    nc.tensor.matmul(
                        ps[half][:, qc * QW:(qc + 1) * QW], lhsT, rhs,
                        start=True, stop=True)
                for half in range(2):
                    sl = slice(qc * QW, (qc + 1) * QW)
                    nc.scalar.activation(es[(half, kt)][:, sl],
                                         ps[half][:, sl],
                                         mybir.ActivationFunctionType.Exp,
                                         scale=SCALE)

            def score_exp_kt(h, qt, kt_, kt, es):
                ps = [ps_score.tile([P, N], F32, tag="score",
                                    name=f"ps_s{h}_{kt}_{half}")
                      for half in range(2)]
                # interleave halves: disjoint PE row groups pack on HW
                for qc in range(QC):
                    for half in range(2):
                        lhsT = kt_[half * DH:(half + 1) * DH,
                                   kt * P:(kt + 1) * P]
                        rhs = qt[half * DH:(half + 1) * DH,
                                 qc * QW:(qc + 1) * QW]
                        nc.tensor.matmul(
                            ps[half][:, qc * QW:(qc + 1) * QW], lhsT, rhs,
                            start=True, stop=True)
                for half in range(2):
                    e = ep.tile([P, N], EDT, tag="e",
                                name=f"e{h}_{kt}_{half}")
                    nc.scalar.activation(e[:], ps[half][:],
                                         mybir.ActivationFunctionType.Exp,
                                         scale=SCALE)
                    es[(half, kt)] = e

            def pv_piece(h, es, us, qc, half, pool=None, tag=None,
                         copy_act=False):
                u = us[half]
                ps = (pool or ps_pv).tile([AUG, QW], F32, tag=tag or "pv",
                                          name=f"ps_pv{h}_{half}_{qc}")
                for kt in range(NT):
                    nc.tensor.matmul(
                        ps[:],
                        vaug[:, kt, h, half:half + AUG],
                        es[(half, kt)][:, qc * QW:(qc + 1) * QW],
                        start=(kt == 0), stop=(kt == NT - 1))
                if copy_act:
                    nc.scalar.activation(u[:, qc * QW:(qc + 1) * QW], ps[:],
                                         mybir.ActivationFunctionType.Copy)
                else:
                    nc.vector.tensor_copy(u[:, qc * QW:(qc + 1) * QW], ps[:])

            ostage = []
            for qt_i in range(NT):
                t = outp.tile([P, CV], F32, tag=f"ost{qt_i}", name=f"ost{qt_i}")
                ostage.append(t)

            def finish_pair(hs, us_pair, qt_i, pool=None, tag=None,
                            use_act=False):
                # transpose u blocks: [h0u1 | h0u2 | h1u1 | h1u2], 65 cols
                tr = (pool or ps_misc).tile([P, 4 * AUG], F32,
                                            tag=tag or "misc",
                                            name=f"ps_tr{hs[0]}_{qt_i}")
                for j, h in enumerate(hs):
                    u1, u2 = us_pair[h]
                    nc.tensor.transpose(
                        tr[:, (2 * j) * AUG:(2 * j + 1) * AUG],
                        u1[0:AUG, qt_i * P:(qt_i + 1) * P],
                        ident[0:AUG, 0:AUG])
                    nc.tensor.transpose(
                        tr[:, (2 * j + 1) * AUG:(2 * j + 2) * AUG],
                        u2[0:AUG, qt_i * P:(qt_i + 1) * P],
                        ident[0:AUG, 0:AUG])
                # denominators at cols {0, 129, 130, 259} = [(130,2),(129,2)]
                rr = smallp.tile([P, 4], F32, tag="rr",
                                 name=f"rr_{hs[0]}{qt_i}")
                den = tr[:].rearrange(
                    "p (a b) -> p a b", b=2 * AUG)[:, :, ::2 * AUG - 1]
                nc.vector.reciprocal_approx_fast(
                    rr[:].rearrange("p (a b) -> p a b", b=2), den)
                for j, h in enumerate(hs):
                    o1 = smallp.tile([P, DH], F32, tag="o1",
                                     name=f"o1_{h}{qt_i}")
                    if use_act:
                        nc.scalar.activation(
                            o1[:],
                            tr[:, (2 * j) * AUG + 1:(2 * j) * AUG + AUG],
                            mybir.ActivationFunctionType.Copy,
                            scale=rr[:, 2 * j:2 * j + 1])
                    else:
                        nc.vector.tensor_scalar_mul(
                            o1[:],
                            tr[:, (2 * j) * AUG + 1:(2 * j) * AUG + AUG],
                            rr[:, 2 * j:2 * j + 1])
                    nc.vector.scalar_tensor_tensor(
                        ostage[qt_i][:, h * DH:(h + 1) * DH],
                        tr[:, (2 * j + 1) * AUG:(2 * j + 1) * AUG + DH],
                        rr[:, 2 * j + 1:2 * j + 2],
                        o1[:],
                        op0=mybir.AluOpType.mult,
                        op1=mybir.AluOpType.add)

            # ---------------- schedule ----------------
            # head-0 ramp: qc0 proj chunks -> first score MMs + split exps
            # while the qc1 chunks are still projecting
            qt0 = qk.tile([P, N], QKDT, tag="qt", name="qt0")
            kt0 = qk.tile([P, N], QKDT, tag="kt", name="kt0")
            es0 = {}
            proj_chunk(0, wq_t, bq_sb, qt0, "q", 0, ps_misc, "misc")
            proj_chunk(0, wk_t, bk_sb, kt0, "k", 0, ps_pv, "pv")
            RAMP = 2
            for kt in range(RAMP):
                score_mm_qc(0, qt0, kt0, kt, es0, 0)
            proj_chunk(0, wq_t, bq_sb, qt0, "q", 1, ps_misc, "misc")
            proj_chunk(0, wk_t, bk_sb, kt0, "k", 1, ps_pv, "pv")
            for kt in range(RAMP):
                score_mm_qc(0, qt0, kt0, kt, es0, 1)
            heads = {0: (qt0, kt0)}
            all_es = {}
            all_us = {}

            def emit_out_dma(qt_i):
                nc.sync.dma_start(d["o"][qt_i * P:(qt_i + 1) * P, :],
                                  ostage[qt_i][:])

            # pv piece order per head: qc-major so finish can start early
            PIECES = [(0, 0), (0, 1), (1, 0), (1, 1)]

            for h in range(HPC):
                qt, kt_ = heads[h]
                es = es0 if h == 0 else {}
                all_es[h] = es
                if h >= 1:
                    all_us[h - 1] = [
                        up.tile([AUG, N], F32, tag="u", name=f"u{h - 1}_{hf}")
                        for hf in range(2)]
                for kt in range(NT):
                    if not (h == 0 and kt < 2):
                        score_exp_kt(h, qt, kt_, kt, es)
                    if h == 0:
                        # v projection spread through head 0's score phase
                        if kt % 2 == 1:
                            proj_v(kt - 1)
                            proj_v(kt)
                    elif kt % 2 == 1:
                        # one PV piece of head h-1 every other kt
                        qc, half = PIECES[kt // 2]
                        pv_piece(h - 1, all_es[h - 1], all_us[h - 1],
                                 qc, half)
                    # finishes of pair (0,1): qt 0-3 spread over h=2 kt 4-7
                    # (qc0 pieces of pv(1) land by kt=3), qt 4-7 over h=3
                    if h == 2 and kt >= 4:
                        finish_pair((0, 1), all_us, kt - 4)
                    if h == 3 and kt < 4:
                        finish_pair((0, 1), all_us, kt + 4)
                    if kt == 0 and h + 1 < HPC:
                        heads[h + 1] = (proj_head_q(h + 1), None)
                    if kt == 2 and h + 1 < HPC:
                        heads[h + 1] = (heads[h + 1][0], proj_head_k(h + 1))

            # tail: pv of last head, finish last pair qc-pipelined
            h = HPC - 1
            all_us[h] = [up.tile([AUG, N], F32, tag="u", name=f"u{h}_{hf}")
                         for hf in range(2)]
            for qc in range(QC):
                pv_piece(h, all_es[h], all_us[h], qc, 0)
                pv_piece(h, all_es[h], all_us[h], qc, 1,
                         pool=ps_score, tag="score")
                lo, hi = (0, NT // 2) if qc == 0 else (NT // 2, NT)
                for i, qt_i in enumerate(range(lo, hi)):
                    if i % 2 == 0:
                        finish_pair((h - 1, h), all_us, qt_i, use_act=True)
                    else:
                        finish_pair((h - 1, h), all_us, qt_i,
                                    pool=ps_score, tag="score",
                                    use_act=True)
                    emit_out_dma(qt_i)

        if reps == 1:
            body()
        else:
            tc.For_i_unrolled(0, reps, 1, lambda iv: body(), max_unroll=2)

    nc.compile()
    return nc


_NC_CACHE = {}


def get_nc(reps=1):
    if reps not in _NC_CACHE:
        _NC_CACHE[reps] = build_nc(reps)
    return _NC_CACHE[reps]


def shard_inputs(inputs):
    import ml_dtypes
    xw_np = np.dtype(ml_dtypes.bfloat16)
    x = np.asarray(inputs["x"], dtype=np.float32)
    Wq = np.asarray(inputs["Wq"], dtype=np.float32)
    bq = np.asarray(inputs["bq"], dtype=np.float32)
    Wk = np.asarray(inputs["Wk"], dtype=np.float32)
    bk = np.asarray(inputs["bk"], dtype=np.float32)
    Wv = np.asarray(inputs["Wv"], dtype=np.float32)
    bv = np.asarray(inputs["bv"], dtype=np.float32)
    in_maps = []
    for c in range(NCORES):
        b = c // 2
        h0 = (c % 2) * HPC
        cq0 = h0 * 2 * DH
        cv0 = h0 * DH
        bias = np.zeros((P, NB), dtype=np.float32)
        bias[:, 0:HPC] = bq[cq0:cq0 + CQ].reshape(HPC, P).T
        bias[:, HPC:2 * HPC] = bk[cq0:cq0 + CQ].reshape(HPC, P).T
        bias[:, 2 * HPC:] = np.broadcast_to(bv[cv0:cv0 + CV], (P, CV))
        in_maps.append({
            "xt": np.ascontiguousarray(x[b].T).astype(xw_np),
            "wq": np.ascontiguousarray(Wq[:, cq0:cq0 + CQ]).astype(xw_np),
            "wk": np.ascontiguousarray(Wk[:, cq0:cq0 + CQ]).astype(xw_np),
            "wv": np.ascontiguousarray(Wv[:, cv0:cv0 + CV]).astype(xw_np),
            "bias": bias,
        })
    return in_maps


def assemble_output(results):
    out = np.empty((B, N, D), dtype=np.float32)
    for c in range(NCORES):
        b = c // 2
        g = c % 2
        out[b, :, g * CV:(g + 1) * CV] = results[c]["o"]
    return out


def kernel(**inputs):
    nc = get_nc(1)
    in_maps = shard_inputs(inputs)
    res = bass_utils.run_bass_kernel_spmd(
        nc, in_maps, core_ids=list(range(NCORES)))
    return assemble_output(res.results)
